# revision 38
# baseline (speedup 1.0000x reference)
"""Trainium2 Bass kernel for the GRU caption model.

Computes: h0 = feat @ W_hp.T + b_hp; 200-step GRU with constant hidden-proj
gate pre-activations; logits = outs @ W_out.T + b_out -> [B, V, T].

Strategy: every core runs the (tiny, latency-bound) GRU redundantly; the
vocab dimension of W_out is sharded 8 ways; each core emits its own
[B, 3840, T] logits slice which the host concatenates.

Schedule: GRU steps are grouped into 8-step chunks. While the GRU runs
chunk c+1, the vocab projection for chunk c's hidden states is interleaved
into the PE queue a few matmuls per step, keeping the tensor engine
continuously busy (max p-state) and hiding the recurrence's cross-engine
dependency-chain latency. Output DMAs use 1024B contiguous runs; the host
unscrambles the [MT, 128, NCH, B, TC] layout for free.

The constant gate pre-activations (C_rz = gh_rz + b_ih_rz and
E_n = 0.5*gh_n + b_ih_n) are preloaded into PSUM via identity matmuls
(bf16 hi+lo, exact to ~2^-17) so the gate matmuls accumulate on top of
them and tanh reads PSUM directly. The r/z/n gate groups live in three
bank-aligned PSUM tiles so the r-gate tanh fires as soon as the r matmuls
land instead of waiting for the whole gate tile.
"""

import numpy as np
import ml_dtypes

import concourse.bass as bass
import concourse.mybir as mybir
import concourse.tile as tile
from concourse import bacc
from concourse.bass_utils import run_bass_kernel_spmd

F32 = mybir.dt.float32
BF16 = mybir.dt.bfloat16
FP8 = mybir.dt.float8e4
AF = mybir.ActivationFunctionType
ALU = mybir.AluOpType
DR = mybir.MatmulPerfMode.DoubleRow

PSCALE = 8.0           # fp8 pre-scale for W_out and res (epilogue undoes it)

VOCAB = 30522
HID = 512
FEAT = 2048
STEPS = 200
BATCH = 32
SOS = 101
NCORES = 8
P = 128
KO = HID // P          # 4 h-chunks
GM = 3 * HID // P      # 12 gate row-groups (r: 0-3, z: 4-7, n: 8-11)
KF = FEAT // P         # 16 feat chunks
VPAD = 3840            # per-core padded vocab rows = 30 * 128
MT = VPAD // P         # 30 vocab tiles per core
TC = 8                 # GRU steps per projection chunk
NCH = STEPS // TC      # 25 chunks
NPROJ = BATCH * TC     # moving free size per proj matmul = 256

LAST_RESULTS = None    # test harness introspection
UNITS_PER_STEP = 4     # proj units interleaved per GRU step


def build():
    nc = bacc.Bacc("TRN2", target_bir_lowering=False, debug=False)

    featT = nc.dram_tensor("featT", [FEAT, BATCH], F32, kind="ExternalInput")
    WhpT = nc.dram_tensor("WhpT", [FEAT, HID], F32, kind="ExternalInput")
    WihT = nc.dram_tensor("WihT", [HID, 3 * HID], BF16, kind="ExternalInput")
    WhhT = nc.dram_tensor("WhhT", [HID, 3 * HID], F32, kind="ExternalInput")
    b_ih = nc.dram_tensor("b_ih", [3 * HID], F32, kind="ExternalInput")
    b_hh = nc.dram_tensor("b_hh", [3 * HID], F32, kind="ExternalInput")
    b_hp = nc.dram_tensor("b_hp", [HID], F32, kind="ExternalInput")
    x0T = nc.dram_tensor("x0T", [HID, BATCH], BF16, kind="ExternalInput")
    I128 = nc.dram_tensor("I128", [P, P], BF16, kind="ExternalInput")
    Wo8hi = nc.dram_tensor("Wo8hi", [HID, VPAD], FP8, kind="ExternalInput")
    Wo8lo = nc.dram_tensor("Wo8lo", [HID, VPAD], FP8, kind="ExternalInput")
    b_out = nc.dram_tensor("b_out", [VPAD], F32, kind="ExternalInput")
    OUT = nc.dram_tensor("OUT", [MT, P, NCH, BATCH, TC], F32, kind="ExternalOutput")

    with tile.TileContext(nc) as tc:
        with (
            tc.tile_pool(name="const", bufs=1) as const,
            tc.tile_pool(name="stream", bufs=3) as stream,
            tc.tile_pool(name="step", bufs=3) as sp,
            tc.tile_pool(name="hb", bufs=3) as hb,
            tc.tile_pool(name="outp", bufs=6) as outp,
            tc.tile_pool(name="psg", bufs=2, space="PSUM") as psg,
            tc.tile_pool(name="psp", bufs=2, space="PSUM") as psp,
        ):
            # ---- constants into SBUF (order = DMA queue order; the h0/gh
            # weight streams are emitted inside the loops below, and the big
            # wih/wout loads are deferred until after them so they don't
            # block the startup-critical transfers) ----
            featT_sb = const.tile([P, KF, BATCH], F32, tag="featsb")
            nc.sync.dma_start(featT_sb[:], featT.rearrange("(k p) b -> p k b", p=P))
            bih_sb = const.tile([P, GM], F32, tag="bih")
            nc.sync.dma_start(bih_sb[:], b_ih.rearrange("(m p) -> p m", p=P))
            bhh_sb = const.tile([P, GM], F32, tag="bhh")
            nc.sync.dma_start(bhh_sb[:], b_hh.rearrange("(m p) -> p m", p=P))
            bhp_sb = const.tile([P, KO], F32, tag="bhp")
            nc.sync.dma_start(bhp_sb[:], b_hp.rearrange("(m p) -> p m", p=P))
            x0_sb = const.tile([P, KO, BATCH], BF16, tag="x0")
            nc.sync.dma_start(x0_sb[:], x0T.rearrange("(k p) b -> p k b", p=P))
            i128_sb = const.tile([P, P], BF16, tag="i128")
            nc.sync.dma_start(i128_sb[:], I128[:, :])
            halves = const.tile([P, KO, BATCH], F32, tag="halves")
            nc.vector.memset(halves[:], 0.5)

            whp_sb = const.tile([P, KF, HID], F32, tag="whp")
            nc.sync.dma_start(whp_sb[:], WhpT.rearrange("(k p) h -> p k h", p=P))
            whh_sb = const.tile([P, KO, 3 * HID], F32, tag="whh")
            nc.sync.dma_start(whh_sb[:], WhhT.rearrange("(k p) g -> p k g", p=P))

            # ---- h0 = feat @ W_hp.T + b_hp (fp32, exact) ----
            ps_h0 = psg.tile([P, 512], F32, tag="psr", name="psr")
            ps_h = ps_h0[:, 0 : KO * BATCH].rearrange("p (m b) -> p m b", b=BATCH)
            for ko in range(KO):
                for kf in range(KF):
                    nc.tensor.matmul(
                        ps_h[:, ko, :], whp_sb[:, kf, ko * P:(ko + 1) * P],
                        featT_sb[:, kf, :],
                        start=(kf == 0), stop=(kf == KF - 1),
                    )
            h0T = const.tile([P, KO, BATCH], F32, tag="h0T")
            for ko in range(KO):
                nc.scalar.activation(
                    h0T[:, ko, :], ps_h[:, ko, :], AF.Identity,
                    bias=bhp_sb[:, ko, None], scale=1.0,
                )
            h0_half = const.tile([P, KO, BATCH], F32, tag="h0h")
            nc.scalar.mul(h0_half[:], h0T[:], 0.5)

            # ---- gh = h0 @ W_hh.T + b_hh (fp32, exact; step-invariant) ----
            ghT = const.tile([P, GM, BATCH], F32, tag="ghT")
            for part, tg in ((0, "psz"), (1, "psn"), (2, "psr")):
                ps_g0 = psg.tile([P, 512], F32, tag=tg, name=tg)
                ps_g = ps_g0[:, 0 : KO * BATCH].rearrange("p (m b) -> p m b", b=BATCH)
                for mi in range(KO):
                    m = part * KO + mi
                    for k in range(KO):
                        nc.tensor.matmul(
                            ps_g[:, mi, :], whh_sb[:, k, m * P:(m + 1) * P],
                            h0T[:, k, :],
                            start=(k == 0), stop=(k == KO - 1),
                        )
                for mi in range(KO):
                    m = part * KO + mi
                    nc.scalar.activation(
                        ghT[:, m, :], ps_g[:, mi, :], AF.Identity,
                        bias=bhh_sb[:, m, None], scale=1.0,
                    )

            # big resident weights (loaded while h0/gh compute)
            wih = const.tile([P, KO, GM, P], BF16, tag="wih")
            nc.sync.dma_start(
                wih[:], WihT.rearrange("(k p) (m c) -> p k m c", p=P, c=P)
            )
            bout_sb = const.tile([P, MT], F32, tag="bout")
            nc.sync.dma_start(bout_sb[:], b_out.rearrange("(m p) -> p m", p=P))
            wo8hi = const.tile([P, KO, VPAD], FP8, tag="wo8hi")
            nc.sync.dma_start(wo8hi[:], Wo8hi.rearrange("(k p) v -> p k v", p=P))
            wo8lo = const.tile([P, KO, VPAD], FP8, tag="wo8lo")
            nc.sync.dma_start(wo8lo[:], Wo8lo.rearrange("(k p) v -> p k v", p=P))

            # C_all = [C_rz ; E_n]: constant additive gate pre-activations
            C_all = const.tile([P, GM, BATCH], F32, tag="Call")
            nc.vector.tensor_add(
                C_all[:, 0:8, :], ghT[:, 0:8, :],
                bih_sb[:, 0:8, None].to_broadcast((P, 8, BATCH)),
            )
            hn2 = const.tile([P, KO, BATCH], F32, tag="hn2")
            nc.scalar.mul(hn2[:], ghT[:, 8:12, :], 0.5)
            nc.vector.tensor_add(
                C_all[:, 8:12, :], hn2[:],
                bih_sb[:, 8:12, None].to_broadcast((P, KO, BATCH)),
            )
            # split into bf16 hi+lo for exact PE psum preload
            C_hi = const.tile([P, GM, BATCH], BF16, tag="Chi")
            nc.vector.tensor_copy(C_hi[:], C_all[:])
            C_hi32 = const.tile([P, GM, BATCH], F32, tag="Chi32")
            nc.scalar.copy(C_hi32[:], C_hi[:])
            C_lo32 = const.tile([P, GM, BATCH], F32, tag="Clo32")
            nc.vector.tensor_sub(C_lo32[:], C_all[:], C_hi32[:])
            C_lo = const.tile([P, GM, BATCH], BF16, tag="Clo")
            nc.vector.tensor_copy(C_lo[:], C_lo32[:])

            # hidden-state history for the projection: fp8 hi+lo (x PSCALE),
            # chunked by TC steps. (The recurrence itself reads the previous
            # step's h from a small bf16 ring.)
            res8h = [
                const.tile([P, KO, BATCH, TC], FP8, tag=f"r8h{c}", name=f"r8h{c}")
                for c in range(NCH)
            ]
            res8l = [
                const.tile([P, KO, BATCH, TC], FP8, tag=f"r8l{c}", name=f"r8l{c}")
                for c in range(NCH)
            ]

            # ---- projection unit: one vocab tile m of chunk c ----
            # logits*64 = (r8h + r8l) @ Wo8hi + r8h @ Wo8lo  (double-row fp8)
            def proj_unit(c, m):
                ps2 = psp.tile([P, NPROJ], F32, tag="pp", name="pp")
                terms = [(wo8hi, res8h[c]), (wo8hi, res8l[c]), (wo8lo, res8h[c])]
                for i, (w8, r8) in enumerate(terms):
                    for pr in range(KO // 2):
                        nc.tensor.matmul(
                            ps2,
                            w8[:, 2 * pr:2 * pr + 2, m * P:(m + 1) * P],
                            r8[:, 2 * pr:2 * pr + 2, :, :],
                            start=(i == 0 and pr == 0),
                            stop=(i == 2 and pr == KO // 2 - 1),
                            perf_mode=DR,
                        )
                ob = outp.tile([P, NPROJ], F32, tag="ob", name="ob")
                eng = nc.gpsimd if m % 2 == 0 else nc.vector
                eng.scalar_tensor_tensor(
                    ob, ps2, 1.0 / (PSCALE * PSCALE),
                    bout_sb[:, m, None].to_broadcast((P, NPROJ)),
                    ALU.mult, ALU.add,
                )
                nc.sync.dma_start(
                    OUT[m, :, c, :, :], ob.rearrange("p (b t) -> p b t", b=BATCH)
                )

            pending = []     # (c, m) proj units ready to emit
            pend_i = 0

            def emit_pending(limit):
                nonlocal pend_i
                done = 0
                while pend_i < len(pending) and done < limit:
                    proj_unit(*pending[pend_i])
                    pend_i += 1
                    done += 1

            def gate_psums():
                """Allocate r/z/n psum tiles and preload C (start=True)."""
                tiles = []
                for i, tg in enumerate(("psr", "psz", "psn")):
                    ps = psg.tile([P, 512], F32, tag=tg, name=tg)
                    flat = ps[:, 0 : KO * BATCH]
                    sl = slice(i * KO, (i + 1) * KO)
                    nc.tensor.matmul(
                        flat, i128_sb[:], C_hi[:, sl, :], start=True, stop=False
                    )
                    nc.tensor.matmul(
                        flat, i128_sb[:], C_lo[:, sl, :], start=False, stop=False
                    )
                    tiles.append(flat.rearrange("p (m b) -> p m b", b=BATCH))
                return tiles

            # ---- GRU steps ----
            cur = gate_psums()
            prev = x0_sb
            for t in range(STEPS):
                c, ti = t // TC, t % TC
                for m in range(GM):
                    ps = cur[m // KO]
                    for k in range(KO):
                        nc.tensor.matmul(
                            ps[:, m % KO, :], wih[:, k, m, :], prev[:, k, :],
                            start=False,
                            stop=(k == KO - 1 and m % KO == KO - 1),
                        )
                ps_r, ps_z, ps_n = cur
                nxt = gate_psums()  # preload next step early (no deps on h)

                t_r = sp.tile([P, KO, BATCH], F32, tag="tr")
                nc.scalar.activation(t_r, ps_r[:, :, :], AF.Tanh, scale=0.5)
                t_z = sp.tile([P, KO, BATCH], F32, tag="tz")
                nc.scalar.activation(t_z, ps_z[:, :, :], AF.Tanh, scale=0.5)
                # a = tanh(0.5 s_r) * 0.5*gh_n  (the r-gate modulation);
                # on DVE so sn2 follows back-to-back on the same engine
                a = sp.tile([P, KO, BATCH], F32, tag="a")
                nc.vector.tensor_mul(a, t_r, hn2)
                # sn2 = gi_n + E_n + a   (E_n already in psum)
                sn2 = sp.tile([P, KO, BATCH], F32, tag="sn2")
                nc.vector.tensor_add(sn2, ps_n[:, :, :], a)
                n = sp.tile([P, KO, BATCH], F32, tag="n")
                nc.scalar.activation(n, sn2, AF.Tanh, scale=1.0)
                # h = (0.5 - 0.5 t_z) * n + (h0/2 + 0.5 t_z * h0)
                c1 = sp.tile([P, KO, BATCH], F32, tag="c1")
                nc.gpsimd.scalar_tensor_tensor(
                    c1, t_z, -0.5, halves[:], ALU.mult, ALU.add
                )
                th0 = sp.tile([P, KO, BATCH], F32, tag="th0")
                nc.gpsimd.scalar_tensor_tensor(
                    th0, t_z, 0.5, h0T[:], ALU.mult, ALU.mult
                )
                c2 = sp.tile([P, KO, BATCH], F32, tag="c2")
                nc.gpsimd.tensor_add(c2, th0, h0_half[:])
                m1 = sp.tile([P, KO, BATCH], F32, tag="m1")
                nc.vector.tensor_mul(m1, c1, n)
                hq = hb.tile([P, KO, BATCH], BF16, tag="hb")
                nc.vector.tensor_add(hq, m1, c2)
                # fp8 hi+lo copies (x PSCALE) for the projection (off-path)
                r8h_t = res8h[c][:, :, :, ti]
                nc.scalar.activation(r8h_t, hq, AF.Identity, scale=PSCALE)
                hi32 = sp.tile([P, KO, BATCH], F32, tag="hi32")
                nc.vector.tensor_copy(hi32, r8h_t)
                nc.vector.scalar_tensor_tensor(
                    res8l[c][:, :, :, ti], hq, PSCALE, hi32,
                    ALU.mult, ALU.subtract,
                )
                prev = hq
                cur = nxt

                # interleave projection work for the previous chunk
                emit_pending(UNITS_PER_STEP)
                if ti == TC - 1:
                    pending.extend((c, m) for m in range(MT))

            # drain the last chunk's projection
            emit_pending(len(pending))

    nc.compile()
    return nc


def _shard_inputs(feat, W_hp, b_hp, W_ih, W_hh, b_ih, b_hh, embed, W_out, b_out):
    bf = ml_dtypes.bfloat16
    featT = np.ascontiguousarray(feat.T, dtype=np.float32)
    WhpT = np.ascontiguousarray(W_hp.T, dtype=np.float32)
    WihT = np.ascontiguousarray(W_ih.T).astype(bf)
    WhhT = np.ascontiguousarray(W_hh.T, dtype=np.float32)
    x0T = np.ascontiguousarray(
        np.repeat(np.asarray(embed)[SOS][:, None], BATCH, axis=1)
    ).astype(bf)
    Wo = np.zeros((NCORES * VPAD, HID), np.float32)
    Wo[:VOCAB] = W_out
    bo = np.zeros((NCORES * VPAD,), np.float32)
    bo[:VOCAB] = b_out
    f8 = ml_dtypes.float8_e4m3fn
    Wo8hi = (Wo * PSCALE).astype(f8)
    Wo8lo = (Wo * PSCALE - Wo8hi.astype(np.float32)).astype(f8)
    common = dict(
        featT=featT, WhpT=WhpT, WihT=WihT, WhhT=WhhT,
        b_ih=np.asarray(b_ih, np.float32), b_hh=np.asarray(b_hh, np.float32),
        b_hp=np.asarray(b_hp, np.float32), x0T=x0T,
        I128=np.eye(P, dtype=np.float32).astype(bf),
    )
    in_maps = []
    for c in range(NCORES):
        sl = slice(c * VPAD, (c + 1) * VPAD)
        m = dict(common)
        m["Wo8hi"] = np.ascontiguousarray(Wo8hi[sl].T)
        m["Wo8lo"] = np.ascontiguousarray(Wo8lo[sl].T)
        m["b_out"] = bo[sl].copy()
        in_maps.append(m)
    return in_maps


def kernel(**inputs):
    global LAST_RESULTS
    args = {k: np.asarray(v) for k, v in inputs.items()}
    in_maps = _shard_inputs(
        args["feat"], args["W_hp"], args["b_hp"], args["W_ih"], args["W_hh"],
        args["b_ih"], args["b_hh"], args["embed"], args["W_out"], args["b_out"],
    )
    nc = build()
    res = run_bass_kernel_spmd(nc, in_maps, core_ids=list(range(NCORES)))
    LAST_RESULTS = res
    # OUT per core: [MT, P, NCH, B, TC] -> [B, VPAD, T]
    parts = []
    for r in res.results:
        o = r["OUT"]  # [30, 128, 25, 32, 8]
        o = o.transpose(3, 0, 1, 2, 4).reshape(BATCH, VPAD, STEPS)
        parts.append(o)
    out = np.concatenate(parts, axis=1)[:, :VOCAB, :]
    return np.ascontiguousarray(out, dtype=np.float32)


# revision 39
# speedup vs baseline: 1.0808x; 1.0808x over previous
"""Trainium2 Bass kernel for the GRU caption model.

Computes: h0 = feat @ W_hp.T + b_hp; 200-step GRU with constant hidden-proj
gate pre-activations; logits = outs @ W_out.T + b_out -> [B, V, T].

Strategy: every core runs the (tiny, latency-bound) GRU redundantly; the
vocab dimension of W_out is sharded 8 ways; each core emits its own
[B, 3840, T] logits slice which the host concatenates.

Schedule: GRU steps are grouped into 8-step chunks. While the GRU runs
chunk c+1, the vocab projection for chunk c's hidden states is interleaved
into the PE queue a few matmuls per step, keeping the tensor engine
continuously busy (max p-state) and hiding the recurrence's cross-engine
dependency-chain latency. Output DMAs use 1024B contiguous runs; the host
unscrambles the [MT, 128, NCH, B, TC] layout for free.

The constant gate pre-activations (C_rz = gh_rz + b_ih_rz and
E_n = 0.5*gh_n + b_ih_n) are preloaded into PSUM via identity matmuls
(bf16 hi+lo, exact to ~2^-17) so the gate matmuls accumulate on top of
them and tanh reads PSUM directly. The r/z/n gate groups live in three
bank-aligned PSUM tiles so the r-gate tanh fires as soon as the r matmuls
land instead of waiting for the whole gate tile.
"""

import numpy as np
import ml_dtypes

import concourse.bass as bass
import concourse.mybir as mybir
import concourse.tile as tile
from concourse import bacc
from concourse.bass_utils import run_bass_kernel_spmd

F32 = mybir.dt.float32
BF16 = mybir.dt.bfloat16
FP8 = mybir.dt.float8e4
AF = mybir.ActivationFunctionType
ALU = mybir.AluOpType
DR = mybir.MatmulPerfMode.DoubleRow

PSCALE = 8.0           # fp8 pre-scale for W_out and res (epilogue undoes it)

VOCAB = 30522
HID = 512
FEAT = 2048
STEPS = 200
BATCH = 32
SOS = 101
NCORES = 8
P = 128
KO = HID // P          # 4 h-chunks
GM = 3 * HID // P      # 12 gate row-groups (r: 0-3, z: 4-7, n: 8-11)
KF = FEAT // P         # 16 feat chunks
VPAD = 3840            # per-core padded vocab rows = 30 * 128
MT = VPAD // P         # 30 vocab tiles per core
TC = 8                 # GRU steps per projection chunk
NCH = STEPS // TC      # 25 chunks
NPROJ = BATCH * TC     # moving free size per proj matmul = 256

LAST_RESULTS = None    # test harness introspection
UNITS_PER_STEP = 4     # proj units interleaved per GRU step


def build():
    nc = bacc.Bacc("TRN2", target_bir_lowering=False, debug=False)

    featT = nc.dram_tensor("featT", [FEAT, BATCH], F32, kind="ExternalInput")
    WhpT = nc.dram_tensor("WhpT", [FEAT, HID], F32, kind="ExternalInput")
    WihT = nc.dram_tensor("WihT", [HID, 3 * HID], BF16, kind="ExternalInput")
    WhhT = nc.dram_tensor("WhhT", [HID, 3 * HID], F32, kind="ExternalInput")
    b_ih = nc.dram_tensor("b_ih", [3 * HID], F32, kind="ExternalInput")
    b_hh = nc.dram_tensor("b_hh", [3 * HID], F32, kind="ExternalInput")
    b_hp = nc.dram_tensor("b_hp", [HID], F32, kind="ExternalInput")
    x0T = nc.dram_tensor("x0T", [HID, BATCH], BF16, kind="ExternalInput")
    I128 = nc.dram_tensor("I128", [P, P], BF16, kind="ExternalInput")
    Wo8hi = nc.dram_tensor("Wo8hi", [HID, VPAD], FP8, kind="ExternalInput")
    Wo8lo = nc.dram_tensor("Wo8lo", [HID, VPAD], FP8, kind="ExternalInput")
    b_out = nc.dram_tensor("b_out", [VPAD], F32, kind="ExternalInput")
    OUT = nc.dram_tensor("OUT", [MT, P, NCH, BATCH, TC], F32, kind="ExternalOutput")

    with tile.TileContext(nc) as tc:
        with (
            tc.tile_pool(name="const", bufs=1) as const,
            tc.tile_pool(name="stream", bufs=3) as stream,
            tc.tile_pool(name="step", bufs=3) as sp,
            tc.tile_pool(name="hb", bufs=3) as hb,
            tc.tile_pool(name="outp", bufs=6) as outp,
            tc.tile_pool(name="psg", bufs=2, space="PSUM") as psg,
            tc.tile_pool(name="psp", bufs=2, space="PSUM") as psp,
        ):
            # ---- constants into SBUF (order = DMA queue order; the h0/gh
            # weight streams are emitted inside the loops below, and the big
            # wih/wout loads are deferred until after them so they don't
            # block the startup-critical transfers) ----
            featT_sb = const.tile([P, KF, BATCH], F32, tag="featsb")
            nc.sync.dma_start(featT_sb[:], featT.rearrange("(k p) b -> p k b", p=P))
            bih_sb = const.tile([P, GM], F32, tag="bih")
            nc.sync.dma_start(bih_sb[:], b_ih.rearrange("(m p) -> p m", p=P))
            bhh_sb = const.tile([P, GM], F32, tag="bhh")
            nc.sync.dma_start(bhh_sb[:], b_hh.rearrange("(m p) -> p m", p=P))
            bhp_sb = const.tile([P, KO], F32, tag="bhp")
            nc.sync.dma_start(bhp_sb[:], b_hp.rearrange("(m p) -> p m", p=P))
            x0_sb = const.tile([P, KO, BATCH], BF16, tag="x0")
            nc.sync.dma_start(x0_sb[:], x0T.rearrange("(k p) b -> p k b", p=P))
            i128_sb = const.tile([P, P], BF16, tag="i128")
            nc.sync.dma_start(i128_sb[:], I128[:, :])
            halves = const.tile([P, KO, BATCH], F32, tag="halves")
            nc.vector.memset(halves[:], 0.5)

            whp_sb = const.tile([P, KF, HID], F32, tag="whp")
            nc.sync.dma_start(whp_sb[:], WhpT.rearrange("(k p) h -> p k h", p=P))
            whh_sb = const.tile([P, KO, 3 * HID], F32, tag="whh")
            nc.sync.dma_start(whh_sb[:], WhhT.rearrange("(k p) g -> p k g", p=P))

            # ---- h0 = feat @ W_hp.T + b_hp (fp32, exact) ----
            ps_h0 = psg.tile([P, 512], F32, tag="psr", name="psr")
            ps_h = ps_h0[:, 0 : KO * BATCH].rearrange("p (m b) -> p m b", b=BATCH)
            for ko in range(KO):
                for kf in range(KF):
                    nc.tensor.matmul(
                        ps_h[:, ko, :], whp_sb[:, kf, ko * P:(ko + 1) * P],
                        featT_sb[:, kf, :],
                        start=(kf == 0), stop=(kf == KF - 1),
                    )
            h0T = const.tile([P, KO, BATCH], F32, tag="h0T")
            for ko in range(KO):
                nc.scalar.activation(
                    h0T[:, ko, :], ps_h[:, ko, :], AF.Identity,
                    bias=bhp_sb[:, ko, None], scale=1.0,
                )
            h0_half = const.tile([P, KO, BATCH], F32, tag="h0h")
            nc.scalar.mul(h0_half[:], h0T[:], 0.5)

            # ---- gh = h0 @ W_hh.T + b_hh (fp32, exact; step-invariant) ----
            ghT = const.tile([P, GM, BATCH], F32, tag="ghT")
            for part, tg in ((0, "psz"), (1, "psn"), (2, "psr")):
                ps_g0 = psg.tile([P, 512], F32, tag=tg, name=tg)
                ps_g = ps_g0[:, 0 : KO * BATCH].rearrange("p (m b) -> p m b", b=BATCH)
                for mi in range(KO):
                    m = part * KO + mi
                    for k in range(KO):
                        nc.tensor.matmul(
                            ps_g[:, mi, :], whh_sb[:, k, m * P:(m + 1) * P],
                            h0T[:, k, :],
                            start=(k == 0), stop=(k == KO - 1),
                        )
                for mi in range(KO):
                    m = part * KO + mi
                    nc.scalar.activation(
                        ghT[:, m, :], ps_g[:, mi, :], AF.Identity,
                        bias=bhh_sb[:, m, None], scale=1.0,
                    )

            # big resident weights (loaded while h0/gh compute)
            wih = const.tile([P, KO, GM, P], BF16, tag="wih")
            nc.sync.dma_start(
                wih[:], WihT.rearrange("(k p) (m c) -> p k m c", p=P, c=P)
            )
            bout_sb = const.tile([P, MT], F32, tag="bout")
            nc.sync.dma_start(bout_sb[:], b_out.rearrange("(m p) -> p m", p=P))
            wo8hi = const.tile([P, KO, VPAD], FP8, tag="wo8hi")
            nc.sync.dma_start(wo8hi[:], Wo8hi.rearrange("(k p) v -> p k v", p=P))
            wo8lo = const.tile([P, KO, VPAD], FP8, tag="wo8lo")
            nc.sync.dma_start(wo8lo[:], Wo8lo.rearrange("(k p) v -> p k v", p=P))

            # C_all = [C_rz ; E_n]: constant additive gate pre-activations
            C_all = const.tile([P, GM, BATCH], F32, tag="Call")
            nc.vector.tensor_add(
                C_all[:, 0:8, :], ghT[:, 0:8, :],
                bih_sb[:, 0:8, None].to_broadcast((P, 8, BATCH)),
            )
            hn2 = const.tile([P, KO, BATCH], F32, tag="hn2")
            nc.scalar.mul(hn2[:], ghT[:, 8:12, :], 0.5)
            nc.vector.tensor_add(
                C_all[:, 8:12, :], hn2[:],
                bih_sb[:, 8:12, None].to_broadcast((P, KO, BATCH)),
            )
            # split into bf16 hi+lo for exact PE psum preload
            C_hi = const.tile([P, GM, BATCH], BF16, tag="Chi")
            nc.vector.tensor_copy(C_hi[:], C_all[:])
            C_hi32 = const.tile([P, GM, BATCH], F32, tag="Chi32")
            nc.scalar.copy(C_hi32[:], C_hi[:])
            C_lo32 = const.tile([P, GM, BATCH], F32, tag="Clo32")
            nc.vector.tensor_sub(C_lo32[:], C_all[:], C_hi32[:])
            C_lo = const.tile([P, GM, BATCH], BF16, tag="Clo")
            nc.vector.tensor_copy(C_lo[:], C_lo32[:])

            # hidden-state history for the projection: fp8 hi+lo (x PSCALE),
            # chunked by TC steps. (The recurrence itself reads the previous
            # step's h from a small bf16 ring.)
            res8h = [
                const.tile([P, KO, BATCH, TC], FP8, tag=f"r8h{c}", name=f"r8h{c}")
                for c in range(NCH)
            ]
            res8l = [
                const.tile([P, KO, BATCH, TC], FP8, tag=f"r8l{c}", name=f"r8l{c}")
                for c in range(NCH)
            ]

            # ---- projection unit: one vocab tile m of chunk c ----
            # logits*64 = (r8h + r8l) @ Wo8hi + r8h @ Wo8lo  (double-row fp8)
            def proj_unit(c, m):
                ps2 = psp.tile([P, NPROJ], F32, tag="pp", name="pp")
                terms = [(wo8hi, res8h[c]), (wo8hi, res8l[c]), (wo8lo, res8h[c])]
                for i, (w8, r8) in enumerate(terms):
                    for pr in range(KO // 2):
                        nc.tensor.matmul(
                            ps2,
                            w8[:, 2 * pr:2 * pr + 2, m * P:(m + 1) * P],
                            r8[:, 2 * pr:2 * pr + 2, :, :],
                            start=(i == 0 and pr == 0),
                            stop=(i == 2 and pr == KO // 2 - 1),
                            perf_mode=DR,
                        )
                ob = outp.tile([P, NPROJ], F32, tag="ob", name="ob")
                eng = nc.gpsimd if m % 2 == 0 else nc.vector
                eng.scalar_tensor_tensor(
                    ob, ps2, 1.0 / (PSCALE * PSCALE),
                    bout_sb[:, m, None].to_broadcast((P, NPROJ)),
                    ALU.mult, ALU.add,
                )
                nc.sync.dma_start(
                    OUT[m, :, c, :, :], ob.rearrange("p (b t) -> p b t", b=BATCH)
                )

            pending = []     # (c, m) proj units ready to emit
            pend_i = 0

            def emit_pending(limit):
                nonlocal pend_i
                done = 0
                while pend_i < len(pending) and done < limit:
                    proj_unit(*pending[pend_i])
                    pend_i += 1
                    done += 1

            def gate_psums():
                """Allocate r/z/n psum tiles and preload C (start=True)."""
                tiles = []
                for i, tg in enumerate(("psr", "psz", "psn")):
                    ps = psg.tile([P, 512], F32, tag=tg, name=tg)
                    flat = ps[:, 0 : KO * BATCH]
                    sl = slice(i * KO, (i + 1) * KO)
                    nc.tensor.matmul(
                        flat, i128_sb[:], C_hi[:, sl, :], start=True, stop=False
                    )
                    nc.tensor.matmul(
                        flat, i128_sb[:], C_lo[:, sl, :], start=False, stop=False
                    )
                    tiles.append(flat.rearrange("p (m b) -> p m b", b=BATCH))
                return tiles

            # ---- GRU steps ----
            cur = gate_psums()
            prev = x0_sb
            for t in range(STEPS):
                c, ti = t // TC, t % TC
                for m in range(GM):
                    ps = cur[m // KO]
                    for k in range(KO):
                        nc.tensor.matmul(
                            ps[:, m % KO, :], wih[:, k, m, :], prev[:, k, :],
                            start=False,
                            stop=(k == KO - 1 and m % KO == KO - 1),
                        )
                ps_r, ps_z, ps_n = cur
                nxt = gate_psums()  # preload next step early (no deps on h)

                t_r = sp.tile([P, KO, BATCH], F32, tag="tr")
                nc.scalar.activation(t_r, ps_r[:, :, :], AF.Tanh, scale=0.5)
                t_z = sp.tile([P, KO, BATCH], F32, tag="tz")
                nc.scalar.activation(t_z, ps_z[:, :, :], AF.Tanh, scale=0.5)
                # a = tanh(0.5 s_r) * 0.5*gh_n  (the r-gate modulation)
                a = sp.tile([P, KO, BATCH], F32, tag="a")
                nc.gpsimd.tensor_mul(a, t_r, hn2)
                # sn2 = gi_n + E_n + a   (E_n already in psum)
                sn2 = sp.tile([P, KO, BATCH], F32, tag="sn2")
                nc.vector.tensor_add(sn2, ps_n[:, :, :], a)
                n = sp.tile([P, KO, BATCH], F32, tag="n")
                nc.scalar.activation(n, sn2, AF.Tanh, scale=1.0)
                # h = (0.5 - 0.5 t_z) * n + (h0/2 + 0.5 t_z * h0)
                c1 = sp.tile([P, KO, BATCH], F32, tag="c1")
                nc.gpsimd.scalar_tensor_tensor(
                    c1, t_z, -0.5, halves[:], ALU.mult, ALU.add
                )
                th0 = sp.tile([P, KO, BATCH], F32, tag="th0")
                nc.gpsimd.scalar_tensor_tensor(
                    th0, t_z, 0.5, h0T[:], ALU.mult, ALU.mult
                )
                c2 = sp.tile([P, KO, BATCH], F32, tag="c2")
                nc.gpsimd.tensor_add(c2, th0, h0_half[:])
                m1 = sp.tile([P, KO, BATCH], F32, tag="m1")
                nc.vector.tensor_mul(m1, c1, n)
                hq = hb.tile([P, KO, BATCH], BF16, tag="hb")
                nc.vector.tensor_add(hq, m1, c2)
                # fp8 hi+lo copies (x PSCALE) for the projection (off-path)
                r8h_t = res8h[c][:, :, :, ti]
                nc.scalar.activation(r8h_t, hq, AF.Identity, scale=PSCALE)
                hi32 = sp.tile([P, KO, BATCH], F32, tag="hi32")
                nc.vector.tensor_copy(hi32, r8h_t)
                nc.vector.scalar_tensor_tensor(
                    res8l[c][:, :, :, ti], hq, PSCALE, hi32,
                    ALU.mult, ALU.subtract,
                )
                prev = hq
                cur = nxt

                # interleave projection work for the previous chunk
                emit_pending(UNITS_PER_STEP)
                if ti == TC - 1:
                    pending.extend((c, m) for m in range(MT))

            # drain the last chunk's projection
            emit_pending(len(pending))

    nc.compile()
    return nc


def _shard_inputs(feat, W_hp, b_hp, W_ih, W_hh, b_ih, b_hh, embed, W_out, b_out):
    bf = ml_dtypes.bfloat16
    featT = np.ascontiguousarray(feat.T, dtype=np.float32)
    WhpT = np.ascontiguousarray(W_hp.T, dtype=np.float32)
    WihT = np.ascontiguousarray(W_ih.T).astype(bf)
    WhhT = np.ascontiguousarray(W_hh.T, dtype=np.float32)
    x0T = np.ascontiguousarray(
        np.repeat(np.asarray(embed)[SOS][:, None], BATCH, axis=1)
    ).astype(bf)
    Wo = np.zeros((NCORES * VPAD, HID), np.float32)
    Wo[:VOCAB] = W_out
    bo = np.zeros((NCORES * VPAD,), np.float32)
    bo[:VOCAB] = b_out
    f8 = ml_dtypes.float8_e4m3fn
    Wo8hi = (Wo * PSCALE).astype(f8)
    Wo8lo = (Wo * PSCALE - Wo8hi.astype(np.float32)).astype(f8)
    common = dict(
        featT=featT, WhpT=WhpT, WihT=WihT, WhhT=WhhT,
        b_ih=np.asarray(b_ih, np.float32), b_hh=np.asarray(b_hh, np.float32),
        b_hp=np.asarray(b_hp, np.float32), x0T=x0T,
        I128=np.eye(P, dtype=np.float32).astype(bf),
    )
    in_maps = []
    for c in range(NCORES):
        sl = slice(c * VPAD, (c + 1) * VPAD)
        m = dict(common)
        m["Wo8hi"] = np.ascontiguousarray(Wo8hi[sl].T)
        m["Wo8lo"] = np.ascontiguousarray(Wo8lo[sl].T)
        m["b_out"] = bo[sl].copy()
        in_maps.append(m)
    return in_maps


def kernel(**inputs):
    global LAST_RESULTS
    args = {k: np.asarray(v) for k, v in inputs.items()}
    in_maps = _shard_inputs(
        args["feat"], args["W_hp"], args["b_hp"], args["W_ih"], args["W_hh"],
        args["b_ih"], args["b_hh"], args["embed"], args["W_out"], args["b_out"],
    )
    nc = build()
    res = run_bass_kernel_spmd(nc, in_maps, core_ids=list(range(NCORES)))
    LAST_RESULTS = res
    # OUT per core: [MT, P, NCH, B, TC] -> [B, VPAD, T]
    parts = []
    for r in res.results:
        o = r["OUT"]  # [30, 128, 25, 32, 8]
        o = o.transpose(3, 0, 1, 2, 4).reshape(BATCH, VPAD, STEPS)
        parts.append(o)
    out = np.concatenate(parts, axis=1)[:, :VOCAB, :]
    return np.ascontiguousarray(out, dtype=np.float32)


# revision 46
# speedup vs baseline: 1.1046x; 1.0220x over previous
"""Trainium2 Bass kernel for the GRU caption model.

Computes: h0 = feat @ W_hp.T + b_hp; 200-step GRU with constant hidden-proj
gate pre-activations; logits = outs @ W_out.T + b_out -> [B, V, T].

Strategy: every core runs the (tiny, latency-bound) GRU redundantly; the
vocab dimension of W_out is sharded 8 ways; each core emits its own
[B, 3840, T] logits slice which the host concatenates.

Schedule: GRU steps are grouped into 8-step chunks. While the GRU runs
chunk c+1, the vocab projection for chunk c's hidden states is interleaved
into the PE queue a few matmuls per step, keeping the tensor engine
continuously busy (max p-state) and hiding the recurrence's cross-engine
dependency-chain latency. Output DMAs use 1024B contiguous runs; the host
unscrambles the [MT, 128, NCH, B, TC] layout for free.

The constant gate pre-activations (C_rz = gh_rz + b_ih_rz and
E_n = 0.5*gh_n + b_ih_n) are preloaded into PSUM via identity matmuls
(bf16 hi+lo, exact to ~2^-17) so the gate matmuls accumulate on top of
them and tanh reads PSUM directly. The r/z/n gate groups live in three
bank-aligned PSUM tiles so the r-gate tanh fires as soon as the r matmuls
land instead of waiting for the whole gate tile.
"""

import numpy as np
import ml_dtypes

import concourse.bass as bass
import concourse.mybir as mybir
import concourse.tile as tile
from concourse import bacc
from concourse.bass_utils import run_bass_kernel_spmd

F32 = mybir.dt.float32
BF16 = mybir.dt.bfloat16
FP16 = mybir.dt.float16
FP8 = mybir.dt.float8e4
AF = mybir.ActivationFunctionType
ALU = mybir.AluOpType
DR = mybir.MatmulPerfMode.DoubleRow

PSCALE = 8.0           # fp8 pre-scale for W_out and res (epilogue undoes it)

VOCAB = 30522
HID = 512
FEAT = 2048
STEPS = 200
BATCH = 32
SOS = 101
NCORES = 8
P = 128
KO = HID // P          # 4 h-chunks
GM = 3 * HID // P      # 12 gate row-groups (r: 0-3, z: 4-7, n: 8-11)
KF = FEAT // P         # 16 feat chunks
VPAD = 3840            # per-core padded vocab rows = 30 * 128
MT = VPAD // P         # 30 vocab tiles per core
TC = 8                 # GRU steps per projection chunk
NCH = STEPS // TC      # 25 chunks
NPROJ = BATCH * TC     # moving free size per proj matmul = 256

LAST_RESULTS = None    # test harness introspection
UNITS_PER_STEP = 4     # proj units interleaved per GRU step


def build():
    nc = bacc.Bacc("TRN2", target_bir_lowering=False, debug=False)

    featT = nc.dram_tensor("featT", [FEAT, BATCH], F32, kind="ExternalInput")
    WhpT = nc.dram_tensor("WhpT", [FEAT, HID], F32, kind="ExternalInput")
    WihT = nc.dram_tensor("WihT", [HID, 3 * HID], BF16, kind="ExternalInput")
    WhhT = nc.dram_tensor("WhhT", [HID, 3 * HID], F32, kind="ExternalInput")
    b_ih = nc.dram_tensor("b_ih", [3 * HID], F32, kind="ExternalInput")
    b_hh = nc.dram_tensor("b_hh", [3 * HID], F32, kind="ExternalInput")
    b_hp = nc.dram_tensor("b_hp", [HID], F32, kind="ExternalInput")
    x0T = nc.dram_tensor("x0T", [HID, BATCH], BF16, kind="ExternalInput")
    I128 = nc.dram_tensor("I128", [P, P], FP16, kind="ExternalInput")
    Wo8hi = nc.dram_tensor("Wo8hi", [HID, VPAD], FP8, kind="ExternalInput")
    Wo8lo = nc.dram_tensor("Wo8lo", [HID, VPAD], FP8, kind="ExternalInput")
    b_out = nc.dram_tensor("b_out", [VPAD], F32, kind="ExternalInput")
    OUT = nc.dram_tensor("OUT", [MT, P, NCH, BATCH, TC], F32, kind="ExternalOutput")

    with tile.TileContext(nc) as tc:
        with (
            tc.tile_pool(name="const", bufs=1) as const,
            tc.tile_pool(name="stream", bufs=3) as stream,
            tc.tile_pool(name="step", bufs=3) as sp,
            tc.tile_pool(name="hb", bufs=3) as hb,
            tc.tile_pool(name="outp", bufs=6) as outp,
            tc.tile_pool(name="psg", bufs=2, space="PSUM") as psg,
            tc.tile_pool(name="psp", bufs=2, space="PSUM") as psp,
        ):
            # ---- constants into SBUF (order = DMA queue order; the h0/gh
            # weight streams are emitted inside the loops below, and the big
            # wih/wout loads are deferred until after them so they don't
            # block the startup-critical transfers) ----
            featT_sb = const.tile([P, KF, BATCH], F32, tag="featsb")
            nc.sync.dma_start(featT_sb[:], featT.rearrange("(k p) b -> p k b", p=P))
            bih_sb = const.tile([P, GM], F32, tag="bih")
            nc.sync.dma_start(bih_sb[:], b_ih.rearrange("(m p) -> p m", p=P))
            bhh_sb = const.tile([P, GM], F32, tag="bhh")
            nc.sync.dma_start(bhh_sb[:], b_hh.rearrange("(m p) -> p m", p=P))
            bhp_sb = const.tile([P, KO], F32, tag="bhp")
            nc.sync.dma_start(bhp_sb[:], b_hp.rearrange("(m p) -> p m", p=P))
            x0_sb = const.tile([P, KO, BATCH], BF16, tag="x0")
            nc.sync.dma_start(x0_sb[:], x0T.rearrange("(k p) b -> p k b", p=P))
            i128_sb = const.tile([P, P], FP16, tag="i128")
            nc.sync.dma_start(i128_sb[:], I128[:, :])
            halves = const.tile([P, KO, BATCH], F32, tag="halves")
            nc.vector.memset(halves[:], 0.5)

            whp_sb = const.tile([P, KF, HID], F32, tag="whp")
            WhpT_r = WhpT.rearrange("(k p) h -> p k h", p=P)
            for kf4 in range(4):
                s4 = slice(4 * kf4, 4 * kf4 + 4)
                nc.sync.dma_start(whp_sb[:, s4, :], WhpT_r[:, s4, :])
            whh_sb = const.tile([P, KO, 3 * HID], F32, tag="whh")
            nc.sync.dma_start(whh_sb[:], WhhT.rearrange("(k p) g -> p k g", p=P))

            # ---- h0 = feat @ W_hp.T + b_hp (fp32, exact) ----
            ps_h0 = psg.tile([P, 512], F32, tag="psr", name="psr")
            ps_h = ps_h0[:, 0 : KO * BATCH].rearrange("p (m b) -> p m b", b=BATCH)
            for ko in range(KO):
                for kf in range(KF):
                    nc.tensor.matmul(
                        ps_h[:, ko, :], whp_sb[:, kf, ko * P:(ko + 1) * P],
                        featT_sb[:, kf, :],
                        start=(kf == 0), stop=(kf == KF - 1),
                    )
            h0T = const.tile([P, KO, BATCH], F32, tag="h0T")
            for ko in range(KO):
                nc.scalar.activation(
                    h0T[:, ko, :], ps_h[:, ko, :], AF.Identity,
                    bias=bhp_sb[:, ko, None], scale=1.0,
                )
            h0_half = const.tile([P, KO, BATCH], F32, tag="h0h")
            nc.scalar.mul(h0_half[:], h0T[:], 0.5)

            # ---- gh = h0 @ W_hh.T + b_hh (fp32, exact; step-invariant) ----
            ghT = const.tile([P, GM, BATCH], F32, tag="ghT")
            for part, tg in ((0, "psz"), (1, "psn"), (2, "psr")):
                ps_g0 = psg.tile([P, 512], F32, tag=tg, name=tg)
                ps_g = ps_g0[:, 0 : KO * BATCH].rearrange("p (m b) -> p m b", b=BATCH)
                for mi in range(KO):
                    m = part * KO + mi
                    for k in range(KO):
                        nc.tensor.matmul(
                            ps_g[:, mi, :], whh_sb[:, k, m * P:(m + 1) * P],
                            h0T[:, k, :],
                            start=(k == 0), stop=(k == KO - 1),
                        )
                for mi in range(KO):
                    m = part * KO + mi
                    nc.scalar.activation(
                        ghT[:, m, :], ps_g[:, mi, :], AF.Identity,
                        bias=bhh_sb[:, m, None], scale=1.0,
                    )

            # big resident weights (loaded while h0/gh compute)
            wih = const.tile([P, KO, GM, P], BF16, tag="wih")
            nc.sync.dma_start(
                wih[:], WihT.rearrange("(k p) (m c) -> p k m c", p=P, c=P)
            )
            bout_sb = const.tile([P, MT], F32, tag="bout")
            nc.sync.dma_start(bout_sb[:], b_out.rearrange("(m p) -> p m", p=P))
            wo8hi = const.tile([P, KO, VPAD], FP8, tag="wo8hi")
            nc.sync.dma_start(wo8hi[:], Wo8hi.rearrange("(k p) v -> p k v", p=P))
            wo8lo = const.tile([P, KO, VPAD], FP8, tag="wo8lo")
            nc.sync.dma_start(wo8lo[:], Wo8lo.rearrange("(k p) v -> p k v", p=P))

            # C_all = [C_rz ; E_n]: constant additive gate pre-activations
            C_all = const.tile([P, GM, BATCH], F32, tag="Call")
            nc.vector.tensor_add(
                C_all[:, 0:8, :], ghT[:, 0:8, :],
                bih_sb[:, 0:8, None].to_broadcast((P, 8, BATCH)),
            )
            hn2 = const.tile([P, KO, BATCH], F32, tag="hn2")
            nc.scalar.mul(hn2[:], ghT[:, 8:12, :], 0.5)
            nc.vector.tensor_add(
                C_all[:, 8:12, :], hn2[:],
                bih_sb[:, 8:12, None].to_broadcast((P, KO, BATCH)),
            )
            # fp16 copy for the PE psum preload (rel err ~5e-4, plenty)
            C_hi = const.tile([P, GM, BATCH], FP16, tag="Chi")
            nc.vector.tensor_copy(C_hi[:], C_all[:])

            # hidden-state history for the projection: fp8 hi+lo (x PSCALE),
            # chunked by TC steps. (The recurrence itself reads the previous
            # step's h from a small bf16 ring.)
            res8h = [
                const.tile([P, KO, BATCH, TC], FP8, tag=f"r8h{c}", name=f"r8h{c}")
                for c in range(NCH)
            ]
            res8l = [
                const.tile([P, KO, BATCH, TC], FP8, tag=f"r8l{c}", name=f"r8l{c}")
                for c in range(NCH)
            ]

            # ---- projection unit: one vocab tile m of chunk c ----
            # logits*64 = (r8h + r8l) @ Wo8hi + r8h @ Wo8lo  (double-row fp8)
            def proj_unit(c, m):
                ps2 = psp.tile([P, NPROJ], F32, tag="pp", name="pp")
                terms = [(wo8hi, res8h[c]), (wo8hi, res8l[c]), (wo8lo, res8h[c])]
                for i, (w8, r8) in enumerate(terms):
                    for pr in range(KO // 2):
                        nc.tensor.matmul(
                            ps2,
                            w8[:, 2 * pr:2 * pr + 2, m * P:(m + 1) * P],
                            r8[:, 2 * pr:2 * pr + 2, :, :],
                            start=(i == 0 and pr == 0),
                            stop=(i == 2 and pr == KO // 2 - 1),
                            perf_mode=DR,
                        )
                ob = outp.tile([P, NPROJ], F32, tag="ob", name="ob")
                eng = nc.gpsimd if m % 2 == 0 else nc.vector
                eng.scalar_tensor_tensor(
                    ob, ps2, 1.0 / (PSCALE * PSCALE),
                    bout_sb[:, m, None].to_broadcast((P, NPROJ)),
                    ALU.mult, ALU.add,
                )
                nc.sync.dma_start(
                    OUT[m, :, c, :, :], ob.rearrange("p (b t) -> p b t", b=BATCH)
                )

            pending = []     # (c, m) proj units ready to emit
            pend_i = 0

            def emit_pending(limit):
                nonlocal pend_i
                done = 0
                while pend_i < len(pending) and done < limit:
                    proj_unit(*pending[pend_i])
                    pend_i += 1
                    done += 1

            def gate_psums():
                """Allocate r/z/n psum tiles and preload C (start=True)."""
                tiles = []
                for i, tg in enumerate(("psr", "psz", "psn")):
                    ps = psg.tile([P, 512], F32, tag=tg, name=tg)
                    flat = ps[:, 0 : KO * BATCH]
                    sl = slice(i * KO, (i + 1) * KO)
                    nc.tensor.matmul(
                        flat, i128_sb[:], C_hi[:, sl, :], start=True, stop=False
                    )
                    tiles.append(flat.rearrange("p (m b) -> p m b", b=BATCH))
                return tiles

            # ---- GRU steps ----
            cur = gate_psums()
            prev = x0_sb
            for t in range(STEPS):
                c, ti = t // TC, t % TC
                for m in range(GM):
                    ps = cur[m // KO]
                    for k in range(KO):
                        nc.tensor.matmul(
                            ps[:, m % KO, :], wih[:, k, m, :], prev[:, k, :],
                            start=False,
                            stop=(k == KO - 1 and m % KO == KO - 1),
                        )
                ps_r, ps_z, ps_n = cur
                nxt = gate_psums()  # preload next step early (no deps on h)

                t_r = sp.tile([P, KO, BATCH], F32, tag="tr")
                nc.scalar.activation(t_r, ps_r[:, :, :], AF.Tanh, scale=0.5)
                t_z = sp.tile([P, KO, BATCH], F32, tag="tz")
                nc.scalar.activation(t_z, ps_z[:, :, :], AF.Tanh, scale=0.5)
                # a = tanh(0.5 s_r) * 0.5*gh_n  (the r-gate modulation)
                a = sp.tile([P, KO, BATCH], F32, tag="a")
                nc.gpsimd.tensor_mul(a, t_r, hn2)
                # sn2 = gi_n + E_n + a   (E_n already in psum)
                sn2 = sp.tile([P, KO, BATCH], F32, tag="sn2")
                nc.vector.tensor_add(sn2, ps_n[:, :, :], a)
                n = sp.tile([P, KO, BATCH], F32, tag="n")
                nc.scalar.activation(n, sn2, AF.Tanh, scale=1.0)
                # h = (0.5 - 0.5 t_z) * n + (h0/2 + 0.5 t_z * h0)
                c1 = sp.tile([P, KO, BATCH], F32, tag="c1")
                nc.gpsimd.scalar_tensor_tensor(
                    c1, t_z, -0.5, halves[:], ALU.mult, ALU.add
                )
                th0 = sp.tile([P, KO, BATCH], F32, tag="th0")
                nc.gpsimd.scalar_tensor_tensor(
                    th0, t_z, 0.5, h0T[:], ALU.mult, ALU.mult
                )
                c2 = sp.tile([P, KO, BATCH], F32, tag="c2")
                nc.gpsimd.tensor_add(c2, th0, h0_half[:])
                m1 = sp.tile([P, KO, BATCH], F32, tag="m1")
                nc.vector.tensor_mul(m1, c1, n)
                hq = hb.tile([P, KO, BATCH], BF16, tag="hb")
                nc.vector.tensor_add(hq, m1, c2)
                # fp8 hi+lo copies (x PSCALE) for the projection (off-path)
                r8h_t = res8h[c][:, :, :, ti]
                nc.scalar.activation(r8h_t, hq, AF.Identity, scale=PSCALE)
                hi32 = sp.tile([P, KO, BATCH], F32, tag="hi32")
                nc.vector.tensor_copy(hi32, r8h_t)
                nc.vector.scalar_tensor_tensor(
                    res8l[c][:, :, :, ti], hq, PSCALE, hi32,
                    ALU.mult, ALU.subtract,
                )
                prev = hq
                cur = nxt

                # interleave projection work for the previous chunk
                emit_pending(UNITS_PER_STEP)
                if ti == TC - 1:
                    pending.extend((c, m) for m in range(MT))

            # drain the last chunk's projection
            emit_pending(len(pending))

    nc.compile()
    return nc


def _shard_inputs(feat, W_hp, b_hp, W_ih, W_hh, b_ih, b_hh, embed, W_out, b_out):
    bf = ml_dtypes.bfloat16
    featT = np.ascontiguousarray(feat.T, dtype=np.float32)
    WhpT = np.ascontiguousarray(W_hp.T, dtype=np.float32)
    WihT = np.ascontiguousarray(W_ih.T).astype(bf)
    WhhT = np.ascontiguousarray(W_hh.T, dtype=np.float32)
    x0T = np.ascontiguousarray(
        np.repeat(np.asarray(embed)[SOS][:, None], BATCH, axis=1)
    ).astype(bf)
    Wo = np.zeros((NCORES * VPAD, HID), np.float32)
    Wo[:VOCAB] = W_out
    bo = np.zeros((NCORES * VPAD,), np.float32)
    bo[:VOCAB] = b_out
    f8 = ml_dtypes.float8_e4m3fn
    Wo8hi = (Wo * PSCALE).astype(f8)
    Wo8lo = (Wo * PSCALE - Wo8hi.astype(np.float32)).astype(f8)
    common = dict(
        featT=featT, WhpT=WhpT, WihT=WihT, WhhT=WhhT,
        b_ih=np.asarray(b_ih, np.float32), b_hh=np.asarray(b_hh, np.float32),
        b_hp=np.asarray(b_hp, np.float32), x0T=x0T,
        I128=np.eye(P, dtype=np.float32).astype(np.float16),
    )
    in_maps = []
    for c in range(NCORES):
        sl = slice(c * VPAD, (c + 1) * VPAD)
        m = dict(common)
        m["Wo8hi"] = np.ascontiguousarray(Wo8hi[sl].T)
        m["Wo8lo"] = np.ascontiguousarray(Wo8lo[sl].T)
        m["b_out"] = bo[sl].copy()
        in_maps.append(m)
    return in_maps


def kernel(**inputs):
    global LAST_RESULTS
    args = {k: np.asarray(v) for k, v in inputs.items()}
    in_maps = _shard_inputs(
        args["feat"], args["W_hp"], args["b_hp"], args["W_ih"], args["W_hh"],
        args["b_ih"], args["b_hh"], args["embed"], args["W_out"], args["b_out"],
    )
    nc = build()
    res = run_bass_kernel_spmd(nc, in_maps, core_ids=list(range(NCORES)))
    LAST_RESULTS = res
    # OUT per core: [MT, P, NCH, B, TC] -> [B, VPAD, T]
    parts = []
    for r in res.results:
        o = r["OUT"]  # [30, 128, 25, 32, 8]
        o = o.transpose(3, 0, 1, 2, 4).reshape(BATCH, VPAD, STEPS)
        parts.append(o)
    out = np.concatenate(parts, axis=1)[:, :VOCAB, :]
    return np.ascontiguousarray(out, dtype=np.float32)
